# revision 53
# baseline (speedup 1.0000x reference)
"""Multi-head attention (B=2, S=2048, D=1024, H=16) on 8 TRN2 NeuronCores.

Sharding: tensor-parallel over heads. Each core owns 2 heads (128 feature
channels): Wq/Wk/Wv sliced column-wise (rows of the [out,in] weight), Wo
sliced row-wise. x/y replicated. Each core emits a partial [4096, 1024]
output (its heads pushed through its Wo slice); host sums the 8 partials.

Optimized vs the 200us v1 baseline (timeline-sim 236us -> 202us; A/B
reps-bench measured a further -20us/rep on HW from the row-tiled rb pair +
bf16 V-transposes; remaining PE-busy ~169us serial / ~142us with row-tile
concurrency is the bf16 matmul floor for this decomposition):
  - biases are zero in this problem: all bias-add DVE traffic dropped
    (nonzero biases fall back to an exact numpy path)
  - denominator rider moved FIRST in the attn@V stationary ([1 | 63 zeros |
    V], M=128) so the softmax denominator lands on PSUM partition 0 where
    reciprocal_approx_fast reads it IN PLACE, and the normalize multiply
    reads the attn@V output directly from PSUM rows 64-127 (legal base-64
    access): the v1 ou/den staging copies (~25us DVE) are gone
  - attn@V emission delayed 5 key-tiles behind scores so the previous
    iteration's normalize chain frees o_ps PSUM slots without stalling PE
  - no serial K/V prologue: batch-0 chunk 0 only, chunks 1-3 + V-transposes
    dripped into iteration 0's key-tile slots; batch-1 projection dripped
    into iterations 1-3, its transposes into its 3-4
  - DMA order = critical path (wk, wv, wq, x0-halves, y-halves; cold
    constants after), input DMAs split in halves so the first projection
    matmuls start after half a chunk; Wo output DMA split per 512 columns
  - engine balance: exp on ACT except 3/16 key tiles on DVE via one-op
    bf16 Schraudolph (error cancels in the softmax ratio; dominates the
    9.6e-3 rel err, gate 2e-2); PSUM->SBUF copies split ACT/DVE
  - the two 1/denom partition-broadcast matmuls are K=1: recr rows at
    partitions 0/32 + tile_position row tiling runs them concurrently
  - V transposed in bf16 (1 cyc/row PE transpose-mode vs 2 for f32)
  - tail epilogue: st copies alternate ACT/DVE and out-DMA goes as one
    full-width transfer per q-block (the serial DMA drain was the tail)

Device-side layout (transposed-by-design, no activation transposes):
  - host feeds x^T, y^T  [1024, 4096] bf16
  - Q^T/K^T = (W x^T)        [128 chan, 4096 tok]   (chan on partitions)
  - scores  S^T = K^T.T-slices @ Q^T  -> [kpos, q]  (softmax axis =
    partitions; two heads run concurrently via PE row tiling)
  - E = exp(S^T) unnormalized bf16; denominator rides as the leading "ones"
    column in the V stationary operand of the attn@V matmul
  - O_un^T per head: PSUM row 0 = denominator, rows 64-127 = output
  - normalize with a ones-matmul partition-broadcast of 1/denom
  - out = O_norm^T.T-slices @ Wo^T   (q back on partitions)
"""

import os
import numpy as np
from contextlib import ExitStack

import ml_dtypes

# Problem constants (hardcoded per contract; kernel.py must be self-contained)
B, S, D = 2, 2048, 1024
T = B * S            # 4096 flattened tokens
N_CORES = 8
DC = D // N_CORES    # 128 channels per core
HD = 64              # head dim
NH = DC // HD        # 2 heads per core
SCALE = 1.0 / np.sqrt(HD)  # folded into Wq on host
KT = S // 128        # 16 key tiles per batch
QC = 512             # query chunk (matmul moving N)
NQC = S // QC        # 4 query chunks per batch
VR = 128             # rider-first head block: [1 | 63 zeros | V(64)]
VW = 2 * VR          # 192: per key-tile V layout for both heads
NDI = D // 128       # 8 contraction chunks for the projections

_CACHE = {}


def _get_nc(reps=1):
    key = f"nc{reps}"
    if key in _CACHE:
        return _CACHE[key]
    import concourse.bass as bass
    import concourse.mybir as mybir
    import concourse.tile as tile
    from concourse import bacc

    f32 = mybir.dt.float32
    bf16 = mybir.dt.bfloat16

    nc = bacc.Bacc(
        "TRN2",
        target_bir_lowering=False,
        debug=False,
        enable_asserts=False,
        num_devices=N_CORES,
    )

    xT_d = nc.dram_tensor("xT", [D, T], bf16, kind="ExternalInput").ap()
    yT_d = nc.dram_tensor("yT", [D, T], bf16, kind="ExternalInput").ap()
    wqT_d = nc.dram_tensor("wqT", [128, D], bf16, kind="ExternalInput").ap()
    wkT_d = nc.dram_tensor("wkT", [128, D], bf16, kind="ExternalInput").ap()
    wvT_d = nc.dram_tensor("wvT", [128, D], bf16, kind="ExternalInput").ap()
    woT_d = nc.dram_tensor("woT", [DC, D], bf16, kind="ExternalInput").ap()
    ident_d = nc.dram_tensor("ident", [128, 128], f32, kind="ExternalInput").ap()
    vpat_d = nc.dram_tensor("vpat", [128, (T // 128) * 64], bf16, kind="ExternalInput").ap()
    fones_d = nc.dram_tensor("fones", [1, HD], mybir.dt.float32r, kind="ExternalInput").ap()
    out_d = nc.dram_tensor("out", [T, D], bf16, kind="ExternalOutput").ap()

    with tile.TileContext(nc) as tc, ExitStack() as top:
        persist = top.enter_context(tc.tile_pool(name="persist", bufs=1))

        # ---- persistent SBUF tensors ----
        wq_sb = persist.tile([128, D], bf16, tag="wq")    # [din-part, 8*128 chan]
        wk_sb = persist.tile([128, D], bf16, tag="wk")
        wv_sb = persist.tile([128, D], bf16, tag="wv")
        wo_sb = persist.tile([DC, D], bf16, tag="wo")     # [d-part, 1024 out]
        id_sb = persist.tile([128, 128], f32, tag="ident")
        ones64_sb = persist.tile([33, HD], mybir.dt.float32r, tag="ones64")
        kT_sb = persist.tile([DC, T], bf16, tag="kT")     # [chan, tok]
        vT_sb = persist.tile([DC, T], bf16, tag="vT")     # bf16: 1 cyc/row PE transpose
        idb_sb = persist.tile([128, 128], bf16, tag="identb")
        v_all = persist.tile([128, (T // 128) * VW], bf16, tag="vall")

        # weights pre-rearranged on host to the SBUF layout (2KB lines).
        # DMA order is the critical path: wk/wv/wq first, then the body
        # issues x0 and the y chunks; cold constants (id, vpat, wo, fones)
        # are emitted inside the body after the batch-0 chunks.
        nc.sync.dma_start(wk_sb[:], wkT_d[:])
        nc.sync.dma_start(wv_sb[:], wvT_d[:])
        nc.sync.dma_start(wq_sb[:], wqT_d[:])

        for _rep in range(reps):
            _build_body(nc, tc, mybir, bass, locals())

    nc.compile()
    _CACHE[key] = nc
    return nc


def _build_body(nc, tc, mybir, bass, env):
    f32 = mybir.dt.float32
    f32r = mybir.dt.float32r
    bf16 = mybir.dt.bfloat16
    i16 = mybir.dt.int16
    PSUM = bass.MemorySpace.PSUM
    xT_d, yT_d, out_d = env["xT_d"], env["yT_d"], env["out_d"]
    ident_d, vpat_d, fones_d, woT_d = (env["ident_d"], env["vpat_d"],
                                       env["fones_d"], env["woT_d"])
    first_rep = env["_rep"] == 0
    wq_sb, wk_sb, wv_sb, wo_sb = env["wq_sb"], env["wk_sb"], env["wv_sb"], env["wo_sb"]
    id_sb, ones64_sb = env["id_sb"], env["ones64_sb"]
    idb_sb = env["idb_sb"]
    kT_sb, vT_sb, v_all = env["kT_sb"], env["vT_sb"], env["v_all"]

    Exp = mybir.ActivationFunctionType.Exp
    Copy = mybir.ActivationFunctionType.Copy
    # bf16 Schraudolph exp on DVE: bitcast(int16(EA*s + EB)) ~= e^s (+-4.4%)
    # for a few key tiles per iteration, to offload the ScalarE exp wall.
    EA = 184.6649652337873
    EB = 16248.6
    SCHRAU_KTS = (5, 9, 13)
    NIT = B * NQC

    with ExitStack() as stk:
        io_pool = stk.enter_context(tc.tile_pool(name="io", bufs=4))
        q_pool = stk.enter_context(tc.tile_pool(name="qp", bufs=2))
        e_pool = stk.enter_context(tc.tile_pool(name="e", bufs=9))
        es_pool = stk.enter_context(tc.tile_pool(name="es", bufs=3))
        rec_pool = stk.enter_context(tc.tile_pool(name="rec", bufs=2))
        on_pool = stk.enter_context(tc.tile_pool(name="on", bufs=2))
        st_pool = stk.enter_context(tc.tile_pool(name="st", bufs=4))
        s_pool = stk.enter_context(tc.tile_pool(name="sps", bufs=2, space=PSUM))
        o_pool = stk.enter_context(tc.tile_pool(name="ops", bufs=2, space=PSUM))
        w_pool = stk.enter_context(tc.tile_pool(name="wps", bufs=2, space=PSUM))

        # ---- K/V projections + V transpose helpers ----
        def kv_dma(name, t0, width):
            HG = NDI // 2
            y_io = [io_pool.tile([128, HG * width], bf16, tag="io",
                                 name=f"{name}_{half}") for half in range(2)]
            for half in range(2):
                nc.sync.dma_start(
                    y_io[half][:].rearrange("p (g t) -> p g t", g=HG),
                    yT_d[half * HG * 128:(half + 1) * HG * 128, t0:t0 + width]
                    .rearrange("(g p) t -> p g t", p=128),
                )
            return y_io

        def kv_pair(name, y_io, width, off, u0, act_k):
            # K + V projection groups for 512 tokens; copies: K on ACT when
            # act_k (prologue, ACT idle), else DVE (steady state).
            HG = NDI // 2
            k_ps = w_pool.tile([128, QC], f32, tag="wps", name=f"kps{name}")
            for di in range(NDI):
                nc.tensor.matmul(
                    k_ps[:], wk_sb[:, di * 128:(di + 1) * 128],
                    y_io[di // HG][:, (di % HG) * width + off:(di % HG) * width + off + QC],
                    start=(di == 0), stop=(di == NDI - 1),
                )
            if act_k:
                nc.scalar.activation(kT_sb[:, u0:u0 + QC], k_ps[:], Copy)
            else:
                nc.vector.tensor_copy(kT_sb[:, u0:u0 + QC], k_ps[:])
            v_ps = w_pool.tile([128, QC], f32, tag="wps", name=f"vps{name}")
            for di in range(NDI):
                nc.tensor.matmul(
                    v_ps[:], wv_sb[:, di * 128:(di + 1) * 128],
                    y_io[di // HG][:, (di % HG) * width + off:(di % HG) * width + off + QC],
                    start=(di == 0), stop=(di == NDI - 1),
                )
            nc.vector.tensor_copy(vT_sb[:, u0:u0 + QC], v_ps[:])

        def vtrans(g, act_side):
            tp = w_pool.tile([128, 2 * QC], bf16, tag="wps", name=f"tp{g}")
            nc.tensor.transpose(tp[:, 0:128], vT_sb[:, g * 128:(g + 1) * 128],
                                idb_sb[:])
            for h in range(NH):
                c0 = g * VW + h * VR + 64
                if act_side:
                    nc.scalar.activation(v_all[:, c0:c0 + HD],
                                         tp[:, h * HD:(h + 1) * HD], Copy)
                else:
                    nc.vector.tensor_copy(v_all[:, c0:c0 + HD],
                                          tp[:, h * HD:(h + 1) * HD])

        # batch-1 drip state: chunk DMAs at (it,kt)=(0,2),(1,2); projection
        # pairs at (0,14),(1,5),(1,14),(2,5); transposes spread over its 2-3.
        b1_io = [None, None]

        def b1_dma(c):
            b1_io[c] = kv_dma(f"yio1_{c}", S + c * 2 * QC, 2 * QC)

        def b1_pair(p):
            c, j = divmod(p, 2)
            u0 = S + c * 2 * QC + j * QC
            kv_pair(f"1_{p}", b1_io[c], 2 * QC, j * QC, u0, act_k=False)

        # ---- attention + Wo: software-pipelined emission.
        # Per kt: scores -> exp emitted immediately (keeps ACT saturated),
        # attn@V for kt-1 emitted one step later (PE never blocks on ACT).
        # The previous iteration's normalize/Wo and the next iteration's
        # Q-projection are dripped into fixed kt slots of this iteration.
        def emit_qproj(it):
            b, qc = divmod(it, NQC)
            q0 = b * S + qc * QC
            HG = NDI // 2
            x_io = [io_pool.tile([128, HG * QC], bf16, tag="io",
                                 name=f"xio{it}_{half}") for half in range(2)]
            for half in range(2):
                nc.sync.dma_start(
                    x_io[half][:].rearrange("p (g t) -> p g t", g=HG),
                    xT_d[half * HG * 128:(half + 1) * HG * 128, q0:q0 + QC]
                    .rearrange("(g p) t -> p g t", p=128),
                )
            q_ps = w_pool.tile([128, QC], f32, tag="wps", name=f"qps{it}")
            for di in range(NDI):
                nc.tensor.matmul(
                    q_ps[:], wq_sb[:, di * 128:(di + 1) * 128],
                    x_io[di // HG][:, (di % HG) * QC:(di % HG + 1) * QC],
                    start=(di == 0), stop=(di == NDI - 1),
                )
            q_sb = q_pool.tile([128, QC], bf16, tag="qp", name=f"qsb{it}")
            nc.vector.tensor_copy(q_sb[:], q_ps[:])
            return q_sb

        def epi_rec(ep):
            # denominator rides on PSUM partition 0 (rider-first layout), so
            # reciprocal_approx_fast reads it in place. recr rows 0 / 32 so
            # the two rb broadcasts can row-tile concurrently (K=1 each).
            recr = rec_pool.tile([33, QC], f32r, tag="recr",
                                 name=f"recr{ep['it']}")
            for h in range(NH):
                rec = rec_pool.tile([1, QC], f32, tag="rec",
                                    name=f"rec{ep['it']}_{h}")
                nc.vector.reciprocal_approx_fast(rec[:], ep["o_ps"][h][0:1, :])
                with nc.allow_low_precision(reason="f32r rounding for rb matmul"):
                    nc.vector.tensor_copy(recr[32 * h:32 * h + 1, :], rec[:])
            ep["recr"] = recr

        def epi_norm(ep, h):
            # NOTE: must be emitted BEFORE this iteration's attn@V matmuls on
            # PE: the next o_ps slot request waits on this norm read, and the
            # norm waits on rb — rb behind attn@V would deadlock.
            if h == 0:
                ep["on"] = on_pool.tile([DC, QC], bf16, tag="on",
                                        name=f"on{ep['it']}")
            rb = w_pool.tile([128, QC], f32, tag="wps", name=f"rb{ep['it']}_{h}")
            nc.tensor.matmul(rb[0:HD, :], ones64_sb[32 * h:32 * h + 1, :],
                             ep["recr"][32 * h:32 * h + 1, :],
                             start=True, stop=True,
                             tile_position=(32 * h, 0))
            # DVE tensor_tensor may read at most one PSUM operand; o_ps is
            # the PSUM one, so the broadcast goes through SBUF.
            rb_sb = rec_pool.tile([HD, QC], f32, tag="rbsb",
                                  name=f"rbsb{ep['it']}_{h}")
            nc.vector.tensor_copy(rb_sb[:], rb[0:HD, :])
            nc.vector.tensor_mul(ep["on"][h * HD:(h + 1) * HD, :],
                                 ep["o_ps"][h][64:64 + HD, :],
                                 rb_sb[:])

        def epi_wo(ep, qs, act_half=False):
            st = st_pool.tile([128, D], bf16, tag="st", name=f"st{ep['it']}_{qs}")
            for nn in range(D // QC):
                wp = w_pool.tile([128, QC], f32, tag="wps",
                                 name=f"wp{ep['it']}_{qs}_{nn}")
                nc.tensor.matmul(
                    wp[:], ep["on"][:, qs * 128:(qs + 1) * 128],
                    wo_sb[:, nn * QC:(nn + 1) * QC], start=True, stop=True,
                )
                # tail epilogue alternates ACT/DVE so the wo chain isn't
                # paced by a single engine's copies
                if act_half and nn == 1:
                    nc.scalar.activation(st[:, nn * QC:(nn + 1) * QC], wp[:], Copy)
                else:
                    nc.vector.tensor_copy(st[:, nn * QC:(nn + 1) * QC], wp[:])
                r0 = ep["q0"] + qs * 128
                if not act_half:
                    nc.sync.dma_start(out_d[r0:r0 + 128, nn * QC:(nn + 1) * QC],
                                      st[:, nn * QC:(nn + 1) * QC])
            if act_half:
                # tail: one full-width DMA per q-block (fewer serial drains)
                nc.sync.dma_start(out_d[r0:r0 + 128, :], st[:])

        def epi_all(ep):
            epi_norm(ep, 0)
            epi_norm(ep, 1)
            for qs in range(QC // 128):
                epi_wo(ep, qs, act_half=True)

        # it0's Q path first (ACT hwdge queue: wq then x0), then batch-0 K/V
        # serially (512-token chunks so PE starts after ~1/4 of the y DMA);
        # batch-1 is dripped into iterations 0-3.
        q_next = emit_qproj(0)
        b0_io = {}
        for tc2 in range(4):
            b0_io[tc2] = kv_dma(f"yio0_{tc2}", tc2 * QC, QC)
        kv_pair("0_0", b0_io[0], QC, 0, 0, act_k=True)
        if first_rep:
            # cold constants, after the hot startup DMAs
            nc.sync.dma_start(id_sb[:], ident_d[:])
            nc.vector.tensor_copy(idb_sb[:], id_sb[:])
            v3 = v_all[:].rearrange("p (t c) -> p t c", c=VW)
            vpat_3d = vpat_d[:].rearrange("p (t o) -> p t o", o=64)
            nc.sync.dma_start(v3[:, :, 0:64], vpat_3d)
            nc.sync.dma_start(v3[:, :, VR:VR + 64], vpat_3d)
            nc.sync.dma_start(ones64_sb[0:1, :], fones_d[:])
            nc.sync.dma_start(ones64_sb[32:33, :], fones_d[:])
            nc.sync.dma_start(wo_sb[:], env["woT_d"][:])
        vtrans(0, act_side=False)
        vtrans(1, act_side=True)
        pend = None
        for it in range(NIT):
            b, qc = divmod(it, NQC)
            q0 = b * S + qc * QC
            q_sb = q_next
            qh = [None]
            o_ps = None
            prev = []
            dly = 5
            for kt in range(KT):
                k0 = b * S + kt * 128
                s_ps = s_pool.tile([128, 2 * QC], f32, tag="sps",
                                   name=f"s{it}_{kt}")
                for h in range(NH):
                    nc.tensor.matmul(
                        s_ps[:, h * QC:(h + 1) * QC],
                        kT_sb[h * HD:(h + 1) * HD, k0:k0 + 128],
                        q_sb[h * HD:(h + 1) * HD, :],
                        start=True, stop=True,
                        tile_position=(h * HD, 0),
                    )
                if kt in SCHRAU_KTS:
                    e_t = es_pool.tile([128, 2 * QC], i16, tag="es",
                                       name=f"ei{it}_{kt}")
                    nc.vector.tensor_scalar(e_t[:], s_ps[:], EA, EB,
                                            op0=mybir.AluOpType.mult,
                                            op1=mybir.AluOpType.add)
                    cast = True
                else:
                    e_t = e_pool.tile([128, 2 * QC], bf16, tag="e",
                                      name=f"e{it}_{kt}")
                    nc.scalar.activation(e_t[:], s_ps[:], Exp)
                    cast = False
                # dripped epilogue (it-1) / prologue (it+1)
                if pend is not None:
                    if kt == 1:
                        epi_norm(pend, 0)
                    elif kt == 2:
                        epi_norm(pend, 1)
                    elif kt in (6, 8, 10, 12):
                        epi_wo(pend, (kt - 6) // 2)
                if kt == 13 and it + 1 < NIT:
                    qh[0] = emit_qproj(it + 1)
                # dripped K/V projections + V transposes:
                # it0 finishes batch-0 (chunks 1-3 at kts 0-2, transposes
                # g+2 after each chunk); its 1-3 carry batch-1; b1 tiles
                # 24-31 transpose early in it4 before their attn@V.
                if it == 0:
                    if kt in (0, 1, 2):
                        kv_pair(f"0_{kt + 1}", b0_io[kt + 1], QC, 0,
                                (kt + 1) * QC, act_k=False)
                    g0 = {3: (2, 4), 5: (4, 6), 7: (6, 8), 9: (8, 10),
                          11: (10, 12), 13: (12, 14), 15: (14, 16)}.get(kt)
                    if g0:
                        for g in range(g0[0], g0[1]):
                            vtrans(g, act_side=(g % 2 == 1))
                if it in (1, 2) and kt == 2:
                    b1_dma(it - 1)
                if (it, kt) in ((1, 14), (2, 5), (2, 14), (3, 5)):
                    b1_pair({(1, 14): 0, (2, 5): 1, (2, 14): 2, (3, 5): 3}[(it, kt)])
                if it == 3 and kt in (2, 3, 4, 7, 9, 11, 14, 15):
                    gi = KT + (2, 3, 4, 7, 9, 11, 14, 15).index(kt)
                    vtrans(gi, act_side=(kt % 2 == 1))
                if it == 4 and kt < 8:
                    vtrans(KT + 8 + kt, act_side=(kt % 2 == 1))
                if o_ps is None and kt >= dly:
                    o_ps = [o_pool.tile([64 + HD, QC], f32, tag="ops",
                                        name=f"o{it}_{h}") for h in range(NH)]
                if len(prev) == dly:
                    pe_t, pcast, pkt = prev.pop(0)
                    for h in range(NH):
                        c0 = (b * KT + pkt) * VW + h * VR
                        rhs = pe_t[:, h * QC:(h + 1) * QC]
                        if pcast:
                            rhs = rhs.bitcast(bf16)
                        nc.tensor.matmul(
                            o_ps[h][:], v_all[:, c0:c0 + VR], rhs,
                            start=(pkt == 0), stop=(pkt == KT - 1),
                        )
                prev.append((e_t, cast, kt))
            # trailing attn@V of this iteration
            for pe_t, pcast, pkt in prev:
                for h in range(NH):
                    c0 = (b * KT + pkt) * VW + h * VR
                    rhs = pe_t[:, h * QC:(h + 1) * QC]
                    if pcast:
                        rhs = rhs.bitcast(bf16)
                    nc.tensor.matmul(
                        o_ps[h][:], v_all[:, c0:c0 + VR], rhs,
                        start=(pkt == 0), stop=(pkt == KT - 1),
                    )
            pend = {"it": it, "q0": q0, "o_ps": o_ps, "recr": None}
            epi_rec(pend)
            q_next = qh[0]
        epi_all(pend)


def _wlayout(w):
    # [D, DC] -> [128, D]: row g*128+p, col c  ->  part p, col g*128+c
    bf = ml_dtypes.bfloat16
    return np.ascontiguousarray(
        np.asarray(w).reshape(NDI, 128, DC).transpose(1, 0, 2).reshape(128, D)
    ).astype(bf)


def _prep_in_maps(x, y, Wq, bq, Wk, bk, Wv, bv, Wo):
    bf = ml_dtypes.bfloat16
    xT = np.ascontiguousarray(x.reshape(T, D).T).astype(bf)
    yT = np.ascontiguousarray(y.reshape(T, D).T).astype(bf)
    ident = np.eye(128, dtype=np.float32)
    vpat = np.zeros((128, (T // 128) * 64), dtype=bf)
    vpat[:, ::64] = 1.0
    in_maps = []
    for c in range(N_CORES):
        sl = slice(c * DC, (c + 1) * DC)
        in_maps.append({
            "xT": xT,
            "yT": yT,
            "wqT": _wlayout(Wq[sl].T * SCALE),
            "wkT": _wlayout(Wk[sl].T),
            "wvT": _wlayout(Wv[sl].T),
            "woT": np.ascontiguousarray(Wo[:, sl].T).astype(bf),
            "ident": ident,
            "vpat": vpat,
            "fones": np.ones((1, HD), dtype=np.float32),
        })
    return in_maps


def _run(in_maps, trace=False):
    if os.environ.get("JAX_PLATFORMS", "").strip() == "cpu":
        os.environ.pop("JAX_PLATFORMS")
    nc = _get_nc()
    from concourse.bass_utils import run_bass_kernel_spmd
    return run_bass_kernel_spmd(nc, in_maps, core_ids=list(range(N_CORES)), trace=trace)


def _numpy_fallback(x, y, mask, Wq, bq, Wk, bk, Wv, bv, Wo, bo):
    Bs, Sq, Dm = x.shape
    H = 16
    q = (x @ Wq.T + bq).reshape(Bs, Sq, H, HD)
    k = (y @ Wk.T + bk).reshape(Bs, -1, H, HD)
    v = (y @ Wv.T + bv).reshape(Bs, -1, H, HD)
    score = np.einsum("bqhd,bkhd->bhqk", q, k) / np.sqrt(HD)
    score = score + (1.0 - mask[:, None, :, :]) * -1e9
    score -= score.max(axis=-1, keepdims=True)
    e = np.exp(score)
    attn = e / e.sum(axis=-1, keepdims=True)
    out = np.einsum("bhqk,bkhd->bqhd", attn, v).reshape(Bs, Sq, Dm)
    return (out @ Wo.T + bo).astype(np.float32)


def kernel(x, y, mask, Wq, bq, Wk, bk, Wv, bv, Wo, bo):
    x = np.asarray(x, dtype=np.float32)
    y = np.asarray(y, dtype=np.float32)
    mask = np.asarray(mask, dtype=np.float32)
    Wq = np.asarray(Wq, dtype=np.float32)
    bq = np.asarray(bq, dtype=np.float32)
    Wk = np.asarray(Wk, dtype=np.float32)
    bk = np.asarray(bk, dtype=np.float32)
    Wv = np.asarray(Wv, dtype=np.float32)
    bv = np.asarray(bv, dtype=np.float32)
    Wo = np.asarray(Wo, dtype=np.float32)
    bo = np.asarray(bo, dtype=np.float32)

    if not np.all(mask == 1.0) or bq.any() or bk.any() or bv.any():
        return _numpy_fallback(x, y, mask, Wq, bq, Wk, bk, Wv, bv, Wo, bo)

    in_maps = _prep_in_maps(x, y, Wq, bq, Wk, bk, Wv, bv, Wo)
    res = _run(in_maps, trace=False)
    total = res.results[0]["out"].astype(np.float32).copy()
    for c in range(1, N_CORES):
        total += res.results[c]["out"]
    total += bo
    return total.reshape(B, S, D).astype(np.float32)
